# revision 19
# baseline (speedup 1.0000x reference)
"""BitLinear (BitNet b1.58) kernel for 8x Trainium2 NeuronCores.

y = (round(x * 127/absmax(x)) @ unpack_ternary(weight).T) * weight_scale / (127/absmax(x))

Strategy (column-parallel / tensor-parallel over output features N):
  - Shard packed weight rows (N) across 8 cores; replicate activations.
  - On device: unpack the 2-bit ternary weights once into resident SBUF bf16
    (two's-complement decode: exact), quantize activations per-token to
    int8-valued bf16 (exact), and run the whole GEMM in bf16 with fp32 PSUM
    accumulation (exact: all products/sums are small integers < 2^24).
  - Host concatenates per-core outputs along N.
"""

import sys
import types
import functools

import numpy as np

# ---------------------------------------------------------------------------
# Problem constants (hardcoded; kernel.py must be self-contained)
# ---------------------------------------------------------------------------
B, S, K, N = 2, 2048, 4096, 16384
NCORES = 8
M = B * S                  # 4096 tokens
NSH = N // NCORES          # 2048 output features per core
P = 128
MAGIC = 12582912.0         # 1.5 * 2**23: float32 round-to-nearest-even bias


def _ensure_axon_hooks():
    """The container's antenv lacks axon_hooks; synthesize it so
    run_bass_kernel_spmd(trace=True) can register the NTFF profile hook."""
    if "antenv.axon_hooks" in sys.modules:
        return
    try:
        import antenv
    except ImportError:
        return
    m = types.ModuleType("antenv.axon_hooks")
    holder = [None]
    m.set_axon_ntff_profile_hook = lambda h: holder.__setitem__(0, h)
    m.get_axon_ntff_profile_hook = lambda: holder[0]
    sys.modules["antenv.axon_hooks"] = m
    antenv.axon_hooks = m
    try:
        from trn_agent_boot.trn_boot import _ntff_profile_via_ctypes

        m.set_axon_ntff_profile_hook(
            _ntff_profile_via_ctypes("/opt/axon/libaxon_pjrt.so")
        )
    except Exception:
        pass


@functools.lru_cache(maxsize=4)
def build_program(wsv: float, m_tokens: int = M, nsh: int = NSH, k: int = K):
    """Build the single-core SPMD Bass program.

    wsv: weight_scale[0] (baked as an immediate into the output scale).
    """
    import concourse.bass as bass  # noqa: F401
    import concourse.mybir as mybir
    import concourse.tile as tile
    from concourse import bacc
    from concourse.bass import ds
    from concourse.masks import make_identity

    f32 = mybir.dt.float32
    bf16 = mybir.dt.bfloat16
    fp8 = mybir.dt.float8e4
    i16 = mybir.dt.int16
    AF = mybir.ActivationFunctionType
    OP = mybir.AluOpType
    AX = mybir.AxisListType

    T = k // 1024            # k8-outer tiles of 128 partitions (4)
    J = 8                    # 2-bit lanes per uint16
    K8 = k // 8              # 512
    MT = m_tokens // P       # 32 m-tiles
    NT = nsh // 512          # 4 n-tiles

    nc = bacc.Bacc("TRN2", target_bir_lowering=False, debug=False,
                   num_devices=NCORES)
    x_d = nc.dram_tensor("x", [m_tokens, k], f32, kind="ExternalInput").ap()
    wp_d = nc.dram_tensor("wp", [nsh, K8], i16, kind="ExternalInput").ap()
    out_d = nc.dram_tensor("out", [m_tokens, nsh], f32,
                           kind="ExternalOutput").ap()

    with tile.TileContext(nc) as tc:
        from contextlib import ExitStack

        with ExitStack() as ctx:
            cpool = ctx.enter_context(tc.tile_pool(name="const", bufs=1))
            wpool = ctx.enter_context(tc.tile_pool(name="w", bufs=1))
            u16pool = ctx.enter_context(tc.tile_pool(name="u16", bufs=2))
            tmppool = ctx.enter_context(tc.tile_pool(name="tmp", bufs=3))
            xpool = ctx.enter_context(tc.tile_pool(name="x", bufs=2))
            xqpool = ctx.enter_context(tc.tile_pool(name="xq", bufs=2))
            xtpool = ctx.enter_context(tc.tile_pool(name="xt", bufs=3))
            opool = ctx.enter_context(tc.tile_pool(name="o", bufs=3))
            spool = ctx.enter_context(tc.tile_pool(name="s", bufs=2))
            pst = ctx.enter_context(
                tc.tile_pool(name="pst", bufs=3, space="PSUM"))
            psm = ctx.enter_context(
                tc.tile_pool(name="psm", bufs=2, space="PSUM"))

            ident = cpool.tile([P, P], bf16, name="ident")
            make_identity(nc, ident[:])

            # PE warmup: dependency-free matmuls fill the otherwise-idle
            # window while the first x tile DMAs in and the weights unpack,
            # and push the HAM clock gate to 8/8 (2.4 GHz) before real
            # matmuls start.
            for _ in range(150):
                wps = pst.tile([P, P], f32, name="wps", tag="pst")
                nc.tensor.matmul(wps[:], lhsT=ident[:], rhs=ident[:],
                                 start=True, stop=True)

            # ---------------- weight prep (one-time) ----------------
            # packed u16 [nsh, K8] --transpose--> [K8, nsh] as T tiles of
            # [128, nsh]; partition p of tile t is k8 = 128*t + p.
            # lane j of u16 holds ternary code for k = 8*k8 + j.
            # fp8e4 holds {-1,0,1} exactly; fp8 rhs runs at bf16 PE speed
            # (no DoubleRow) and halves resident-weight SBUF.
            w_sb = [wpool.tile([P, J, nsh], fp8, name=f"wsb{t}")
                    for t in range(T)]
            u16t = []
            for t in range(T):
                u = u16pool.tile([P, nsh], i16, name="u16t", tag="u16t")
                nc.sync.dma_start(u[:], wp_d[:, ds(P * t, P)], transpose=True)
                u16t.append(u)
            # Emit mi=0 activation load/quant before the unpack so the PE
            # can start transposing early while unpack proceeds.

            def load_quant(mi):
                xs = xpool.tile([P, k], f32, name="xs", tag="xs")
                nc.sync.dma_start(xs[:], x_d[ds(P * mi, P), :])
                amax = spool.tile([P, 1], f32, name="amax", tag="amax")
                nc.vector.tensor_reduce(amax[:], xs[:], axis=AX.X, op=OP.max,
                                        apply_absolute_value=True)
                nc.vector.tensor_scalar_max(amax[:], amax[:], 1e-5)
                # s127 = 127 * (1/amax). No engine has an IEEE divide, so
                # this can differ from the reference's 127/amax by ~1 ulp,
                # flipping round(x*s) only when x*s sits within ~1 ulp of a
                # .5 boundary (couple per million values; |out| shift is one
                # quant step).
                rcp = spool.tile([P, 1], f32, name="rcp", tag="rcp")
                nc.vector.reciprocal(rcp[:], amax[:])
                s127 = spool.tile([P, 1], f32, name="s127", tag="s127")
                nc.vector.tensor_scalar_mul(s127[:], rcp[:], 127.0)
                oscale = spool.tile([P, 1], f32, name="oscale", tag="oscale")
                nc.vector.tensor_scalar_mul(oscale[:], amax[:], wsv / 127.0)
                # xq = RNE(x * s127) via the magic-number trick, all on DVE
                # (HW ACT rounds .5 ties differently from numpy; DVE is
                # exact fp32 RNE).
                xr = xpool.tile([P, k], f32, name="xr", tag="xr")
                nc.vector.tensor_scalar(xr[:], xs[:], s127[:], MAGIC,
                                        OP.mult, OP.add)
                xq = xqpool.tile([P, k], bf16, name="xq", tag="xq")
                nc.vector.tensor_scalar_add(xq[:], xr[:], -MAGIC)
                return xq, oscale

            xq0 = load_quant(0)

            # unpack lane j: t = (u << (14-2j)) & 0xC000 puts the 2-bit
            # field at [15:14]; as int16 that is 16384*decode(field) with
            # decode in {-2,-1,0,1} (two's complement). The ACT scaled copy
            # (x * 2^-14 -> fp8) finishes the exact decode.
            for t in range(T):
                for j in range(J):
                    tmp = tmppool.tile([P, nsh], i16, name="tmp", tag="tmp")
                    nc.vector.tensor_scalar(tmp[:], u16t[t][:],
                                            14 - 2 * j, -16384,
                                            OP.logical_shift_left,
                                            OP.bitwise_and)
                    if j % 2 == 0:
                        nc.scalar.mul(w_sb[t][:, j, :], tmp[:], 2.0 ** -14)
                    else:
                        nc.gpsimd.tensor_scalar_mul(w_sb[t][:, j, :], tmp[:],
                                                    2.0 ** -14)

            # ---------------- main loop ----------------
            qcache = xq0
            for mi in range(MT):
                xq, oscale = qcache
                # transpose xq [m, k] -> xqT [k8p, j, t, m] via PE
                xq_r = xq.rearrange("m (t p j) -> m t p j", t=T, j=J)
                xqT = xtpool.tile([P, J, T, P], bf16, name="xqT", tag="xqT")
                for t in range(T):
                    for j in range(J):
                        ps = pst.tile([P, P], f32, name="pst", tag="pst")
                        nc.tensor.matmul(ps[:], lhsT=xq_r[:, t, :, j],
                                         rhs=ident[:], start=True, stop=True)
                        nc.scalar.copy(xqT[:, j, t, :], ps[:])
                if mi + 1 < MT:
                    qcache = load_quant(mi + 1)
                for ni in range(NT):
                    ps = psm.tile([P, 512], f32, name="psm", tag="psm")
                    step = 0
                    for t in range(T):
                        for j in range(J):
                            nc.tensor.matmul(
                                ps[:], lhsT=xqT[:, j, t, :],
                                rhs=w_sb[t][:, j, ds(512 * ni, 512)],
                                start=(step == 0), stop=(step == T * J - 1))
                            step += 1
                    ot = opool.tile([P, 512], f32, name="ot", tag="ot")
                    nc.vector.tensor_scalar_mul(ot[:], ps[:], oscale[:])
                    nc.sync.dma_start(
                        out_d[ds(P * mi, P), ds(512 * ni, 512)], ot[:])

    nc.compile()
    return nc


def kernel(x: np.ndarray, weight: np.ndarray,
           weight_scale: np.ndarray) -> np.ndarray:
    """x: [B,S,K] f32; weight: [N, K//4] uint8 packed ternary;
    weight_scale: [4] f32 (replicated scalar). Returns [B,S,N] f32."""
    _ensure_axon_hooks()
    from concourse import bass_utils

    x2 = np.ascontiguousarray(np.asarray(x).reshape(M, K), dtype=np.float32)
    wp = np.ascontiguousarray(np.asarray(weight, dtype=np.uint8))
    wp16 = wp.view(np.int16)              # [N, K//8] little-endian pairs
    wsv = float(np.asarray(weight_scale).reshape(-1)[0])

    nc = build_program(wsv)
    in_maps = [
        {"x": x2, "wp": np.ascontiguousarray(wp16[c * NSH:(c + 1) * NSH])}
        for c in range(NCORES)
    ]
    res = bass_utils.run_bass_kernel_spmd(
        nc, in_maps, core_ids=list(range(NCORES)))
    out = np.concatenate(
        [res.results[c]["out"] for c in range(NCORES)], axis=1)
    return out.reshape(B, S, N)


# revision 22
# speedup vs baseline: 1.3943x; 1.3943x over previous
"""BitLinear (BitNet b1.58) kernel for 8x Trainium2 NeuronCores.

y = (round(x * 127/absmax(x)) @ unpack_ternary(weight).T) * weight_scale / (127/absmax(x))

Strategy (column-parallel / tensor-parallel over output features N):
  - Shard packed weight rows (N) across 8 cores; replicate activations.
  - On device: unpack the 2-bit ternary weights once into resident SBUF bf16
    (two's-complement decode: exact), quantize activations per-token to
    int8-valued bf16 (exact), and run the whole GEMM in bf16 with fp32 PSUM
    accumulation (exact: all products/sums are small integers < 2^24).
  - Host concatenates per-core outputs along N.
"""

import sys
import types
import functools

import numpy as np

# ---------------------------------------------------------------------------
# Problem constants (hardcoded; kernel.py must be self-contained)
# ---------------------------------------------------------------------------
B, S, K, N = 2, 2048, 4096, 16384
NCORES = 8
M = B * S                  # 4096 tokens
NSH = N // NCORES          # 2048 output features per core
P = 128
MAGIC = 12582912.0         # 1.5 * 2**23: float32 round-to-nearest-even bias


def _ensure_axon_hooks():
    """The container's antenv lacks axon_hooks; synthesize it so
    run_bass_kernel_spmd(trace=True) can register the NTFF profile hook."""
    if "antenv.axon_hooks" in sys.modules:
        return
    try:
        import antenv
    except ImportError:
        return
    m = types.ModuleType("antenv.axon_hooks")
    holder = [None]
    m.set_axon_ntff_profile_hook = lambda h: holder.__setitem__(0, h)
    m.get_axon_ntff_profile_hook = lambda: holder[0]
    sys.modules["antenv.axon_hooks"] = m
    antenv.axon_hooks = m
    try:
        from trn_agent_boot.trn_boot import _ntff_profile_via_ctypes

        m.set_axon_ntff_profile_hook(
            _ntff_profile_via_ctypes("/opt/axon/libaxon_pjrt.so")
        )
    except Exception:
        pass


@functools.lru_cache(maxsize=4)
def build_program(wsv: float, m_tokens: int = M, nsh: int = NSH, k: int = K):
    """Build the single-core SPMD Bass program.

    wsv: weight_scale[0] (baked as an immediate into the output scale).
    """
    import concourse.bass as bass  # noqa: F401
    import concourse.mybir as mybir
    import concourse.tile as tile
    from concourse import bacc
    from concourse.bass import ds
    from concourse.masks import make_identity

    f32 = mybir.dt.float32
    bf16 = mybir.dt.bfloat16
    fp8 = mybir.dt.float8e4
    i16 = mybir.dt.int16
    AF = mybir.ActivationFunctionType
    OP = mybir.AluOpType
    AX = mybir.AxisListType

    T = k // 1024            # k8-outer tiles of 128 partitions (4)
    J = 8                    # 2-bit lanes per uint16
    K8 = k // 8              # 512
    MT = m_tokens // P       # 32 m-tiles
    NT = nsh // 512          # 4 n-tiles

    nc = bacc.Bacc("TRN2", target_bir_lowering=False, debug=False,
                   num_devices=NCORES)
    x_d = nc.dram_tensor("x", [m_tokens, k], f32, kind="ExternalInput").ap()
    wp_d = nc.dram_tensor("wp", [nsh, K8], i16, kind="ExternalInput").ap()
    out_d = nc.dram_tensor("out", [m_tokens, nsh], f32,
                           kind="ExternalOutput").ap()

    with tile.TileContext(nc) as tc:
        from contextlib import ExitStack

        with ExitStack() as ctx:
            cpool = ctx.enter_context(tc.tile_pool(name="const", bufs=1))
            wpool = ctx.enter_context(tc.tile_pool(name="w", bufs=1))
            u16pool = ctx.enter_context(tc.tile_pool(name="u16", bufs=2))
            tmppool = ctx.enter_context(tc.tile_pool(name="tmp", bufs=3))
            xpool = ctx.enter_context(tc.tile_pool(name="x", bufs=2))
            xqpool = ctx.enter_context(tc.tile_pool(name="xq", bufs=2))
            xtpool = ctx.enter_context(tc.tile_pool(name="xt", bufs=3))
            opool = ctx.enter_context(tc.tile_pool(name="o", bufs=3))
            spool = ctx.enter_context(tc.tile_pool(name="s", bufs=2))
            pst = ctx.enter_context(
                tc.tile_pool(name="pst", bufs=3, space="PSUM"))
            psm = ctx.enter_context(
                tc.tile_pool(name="psm", bufs=2, space="PSUM"))

            ident = cpool.tile([P, P], bf16, name="ident")
            make_identity(nc, ident[:])

            # PE warmup: dependency-free matmuls fill the otherwise-idle
            # window while the first x tile DMAs in and the weights unpack,
            # and push the HAM clock gate to 8/8 (2.4 GHz) before real
            # matmuls start.
            wdummy = cpool.tile([P, P], bf16, name="wdummy")
            nc.gpsimd.memset(wdummy[:], 0.0)
            for _ in range(150):
                wps = pst.tile([P, P], f32, name="wps", tag="pst")
                nc.tensor.matmul(wps[:], lhsT=wdummy[:], rhs=ident[:],
                                 start=True, stop=True)

            # ---------------- weight prep (one-time) ----------------
            # packed u16 [nsh, K8] --transpose--> [K8, nsh] as T tiles of
            # [128, nsh]; partition p of tile t is k8 = 128*t + p.
            # lane j of u16 holds ternary code for k = 8*k8 + j.
            # fp8e4 holds {-1,0,1} exactly; fp8 rhs runs at bf16 PE speed
            # (no DoubleRow) and halves resident-weight SBUF.
            w_sb = [wpool.tile([P, J, nsh], fp8, name=f"wsb{t}")
                    for t in range(T)]
            u16t = []
            for t in range(T):
                u = u16pool.tile([P, nsh], i16, name="u16t", tag="u16t")
                nc.sync.dma_start(u[:], wp_d[:, ds(P * t, P)], transpose=True)
                u16t.append(u)
            # Emit mi=0 activation load/quant before the unpack so the PE
            # can start transposing early while unpack proceeds.

            def load_quant(mi):
                xs = xpool.tile([P, k], f32, name="xs", tag="xs")
                nc.sync.dma_start(xs[:], x_d[ds(P * mi, P), :])
                amax = spool.tile([P, 1], f32, name="amax", tag="amax")
                nc.vector.tensor_reduce(amax[:], xs[:], axis=AX.X, op=OP.max,
                                        apply_absolute_value=True)
                nc.vector.tensor_scalar_max(amax[:], amax[:], 1e-5)
                # s127 = 127 * (1/amax). No engine has an IEEE divide, so
                # this can differ from the reference's 127/amax by ~1 ulp,
                # flipping round(x*s) only when x*s sits within ~1 ulp of a
                # .5 boundary (couple per million values; |out| shift is one
                # quant step).
                rcp = spool.tile([P, 1], f32, name="rcp", tag="rcp")
                nc.vector.reciprocal(rcp[:], amax[:])
                s127 = spool.tile([P, 1], f32, name="s127", tag="s127")
                nc.vector.tensor_scalar_mul(s127[:], rcp[:], 127.0)
                oscale = spool.tile([P, 1], f32, name="oscale", tag="oscale")
                nc.vector.tensor_scalar_mul(oscale[:], amax[:], wsv / 127.0)
                # xq = RNE(x * s127) via the magic-number trick, all on DVE
                # (HW ACT rounds .5 ties differently from numpy; DVE is
                # exact fp32 RNE).
                xr = xpool.tile([P, k], f32, name="xr", tag="xr")
                nc.vector.tensor_scalar(xr[:], xs[:], s127[:], MAGIC,
                                        OP.mult, OP.add)
                xq = xqpool.tile([P, k], bf16, name="xq", tag="xq")
                nc.vector.tensor_scalar_add(xq[:], xr[:], -MAGIC)
                return xq, oscale

            xq0 = load_quant(0)

            # unpack lane j: t = (u << (14-2j)) & 0xC000 puts the 2-bit
            # field at [15:14]; as int16 that is 16384*decode(field) with
            # decode in {-2,-1,0,1} (two's complement). The ACT scaled copy
            # (x * 2^-14 -> fp8) finishes the exact decode.
            for t in range(T):
                for j in range(J):
                    tmp = tmppool.tile([P, nsh], i16, name="tmp", tag="tmp")
                    nc.vector.tensor_scalar(tmp[:], u16t[t][:],
                                            14 - 2 * j, -16384,
                                            OP.logical_shift_left,
                                            OP.bitwise_and)
                    nc.scalar.mul(w_sb[t][:, j, :], tmp[:], 2.0 ** -14)

            # ---------------- main loop ----------------
            qcache = xq0
            for mi in range(MT):
                xq, oscale = qcache
                # transpose xq [m, k] -> xqT [k8p, j, t, m] via PE
                xq_r = xq.rearrange("m (t p j) -> m t p j", t=T, j=J)
                xqT = xtpool.tile([P, J, T, P], bf16, name="xqT", tag="xqT")
                for t in range(T):
                    for j in range(J):
                        ps = pst.tile([P, P], f32, name="pst", tag="pst")
                        nc.tensor.matmul(ps[:], lhsT=xq_r[:, t, :, j],
                                         rhs=ident[:], start=True, stop=True)
                        nc.scalar.copy(xqT[:, j, t, :], ps[:])
                if mi + 1 < MT:
                    qcache = load_quant(mi + 1)
                for ni in range(NT):
                    ps = psm.tile([P, 512], f32, name="psm", tag="psm")
                    step = 0
                    for t in range(T):
                        for j in range(J):
                            nc.tensor.matmul(
                                ps[:], lhsT=xqT[:, j, t, :],
                                rhs=w_sb[t][:, j, ds(512 * ni, 512)],
                                start=(step == 0), stop=(step == T * J - 1))
                            step += 1
                    ot = opool.tile([P, 512], f32, name="ot", tag="ot")
                    nc.vector.tensor_scalar_mul(ot[:], ps[:], oscale[:])
                    nc.sync.dma_start(
                        out_d[ds(P * mi, P), ds(512 * ni, 512)], ot[:])

    nc.compile()
    return nc


def kernel(x: np.ndarray, weight: np.ndarray,
           weight_scale: np.ndarray) -> np.ndarray:
    """x: [B,S,K] f32; weight: [N, K//4] uint8 packed ternary;
    weight_scale: [4] f32 (replicated scalar). Returns [B,S,N] f32."""
    _ensure_axon_hooks()
    from concourse import bass_utils

    x2 = np.ascontiguousarray(np.asarray(x).reshape(M, K), dtype=np.float32)
    wp = np.ascontiguousarray(np.asarray(weight, dtype=np.uint8))
    wp16 = wp.view(np.int16)              # [N, K//8] little-endian pairs
    wsv = float(np.asarray(weight_scale).reshape(-1)[0])

    nc = build_program(wsv)
    in_maps = [
        {"x": x2, "wp": np.ascontiguousarray(wp16[c * NSH:(c + 1) * NSH])}
        for c in range(NCORES)
    ]
    last_err = None
    for _attempt in range(3):
        try:
            res = bass_utils.run_bass_kernel_spmd(
                nc, in_maps, core_ids=list(range(NCORES)))
            break
        except Exception as e:  # transient NRT device errors recover on retry
            last_err = e
            import time as _time
            _time.sleep(5.0)
    else:
        raise last_err
    out = np.concatenate(
        [res.results[c]["out"] for c in range(NCORES)], axis=1)
    return out.reshape(B, S, N)


# revision 28
# speedup vs baseline: 1.4496x; 1.0397x over previous
"""BitLinear (BitNet b1.58) kernel for 8x Trainium2 NeuronCores.

y = (round(x * 127/absmax(x)) @ unpack_ternary(weight).T) * weight_scale / (127/absmax(x))

Strategy (column-parallel / tensor-parallel over output features N):
  - Shard packed weight rows (N) across 8 cores; replicate activations.
  - On device: unpack the 2-bit ternary weights once into resident SBUF fp8
    (two's-complement decode: exact), quantize activations per-token to
    int8-valued bf16 (exact), and run the whole GEMM in bf16xfp8 with fp32
    PSUM accumulation (exact: all products/sums are small integers < 2^24).
  - x is uploaded twice: token-major (for the per-token absmax reduce) and
    K-major pre-permuted to the weight partition order (pure host layout
    change) so no on-device transposes are needed; quantization is applied
    directly to the K-major copy with the per-token scale replicated across
    partitions by a pair of tiny DMAs.
  - Host concatenates per-core outputs along N.
"""

import sys
import types
import functools

import numpy as np

# ---------------------------------------------------------------------------
# Problem constants (hardcoded; kernel.py must be self-contained)
# ---------------------------------------------------------------------------
B, S, K, N = 2, 2048, 4096, 16384
NCORES = 8
M = B * S                  # 4096 tokens
NSH = N // NCORES          # 2048 output features per core
P = 128
MAGIC = 12582912.0         # 1.5 * 2**23: float32 round-to-nearest-even bias


def _ensure_axon_hooks():
    """The container's antenv lacks axon_hooks; synthesize it so
    run_bass_kernel_spmd(trace=True) can register the NTFF profile hook."""
    if "antenv.axon_hooks" in sys.modules:
        return
    try:
        import antenv
    except ImportError:
        return
    m = types.ModuleType("antenv.axon_hooks")
    holder = [None]
    m.set_axon_ntff_profile_hook = lambda h: holder.__setitem__(0, h)
    m.get_axon_ntff_profile_hook = lambda: holder[0]
    sys.modules["antenv.axon_hooks"] = m
    antenv.axon_hooks = m
    try:
        from trn_agent_boot.trn_boot import _ntff_profile_via_ctypes

        m.set_axon_ntff_profile_hook(
            _ntff_profile_via_ctypes("/opt/axon/libaxon_pjrt.so")
        )
    except Exception:
        pass


@functools.lru_cache(maxsize=4)
def build_program(wsv: float, m_tokens: int = M, nsh: int = NSH, k: int = K):
    """Build the single-core SPMD Bass program.

    wsv: weight_scale[0] (baked as an immediate into the output scale).
    """
    import concourse.bass as bass  # noqa: F401
    import concourse.mybir as mybir
    import concourse.tile as tile
    from concourse import bacc
    from concourse.bass import ds
    from concourse.masks import make_identity

    f32 = mybir.dt.float32
    bf16 = mybir.dt.bfloat16
    fp8 = mybir.dt.float8e4
    i16 = mybir.dt.int16
    OP = mybir.AluOpType
    AX = mybir.AxisListType

    T = k // 1024            # k8-outer tiles of 128 partitions (4)
    J = 8                    # 2-bit lanes per uint16
    K8 = k // 8              # 512
    MT = m_tokens // P       # m-tiles
    NT = nsh // 512          # n-tiles
    NB = 512                 # unpack column-block size (matches MM groups)

    nc = bacc.Bacc("TRN2", target_bir_lowering=False, debug=False,
                   num_devices=NCORES)
    x_d = nc.dram_tensor("x", [m_tokens, k], f32, kind="ExternalInput").ap()
    # K-major permuted activations: xt[t, p, j, m] = x[m, 1024t + 8p + j]
    xt_d = nc.dram_tensor("xt", [T, P, J, m_tokens], f32,
                          kind="ExternalInput").ap()
    wp_d = nc.dram_tensor("wp", [nsh, K8], i16, kind="ExternalInput").ap()
    out_d = nc.dram_tensor("out", [m_tokens, nsh], f32,
                           kind="ExternalOutput").ap()

    with tile.TileContext(nc) as tc:
        from contextlib import ExitStack

        with ExitStack() as ctx:
            cpool = ctx.enter_context(tc.tile_pool(name="const", bufs=1))
            wpool = ctx.enter_context(tc.tile_pool(name="w", bufs=1))
            u16pool = ctx.enter_context(tc.tile_pool(name="u16", bufs=4))
            tmppool = ctx.enter_context(tc.tile_pool(name="tmp", bufs=3))
            xpool = ctx.enter_context(tc.tile_pool(name="x", bufs=2))
            xtfpool = ctx.enter_context(tc.tile_pool(name="xtf", bufs=2))
            xqpool = ctx.enter_context(tc.tile_pool(name="xq", bufs=2))
            opool = ctx.enter_context(tc.tile_pool(name="o", bufs=3))
            spool = ctx.enter_context(tc.tile_pool(name="s", bufs=2))
            pwarm = ctx.enter_context(
                tc.tile_pool(name="pwarm", bufs=2, space="PSUM"))
            psm = ctx.enter_context(
                tc.tile_pool(name="psm", bufs=3, space="PSUM"))

            ident = cpool.tile([P, P], bf16, name="ident")
            make_identity(nc, ident[:])

            # PE warmup: dependency-free matmuls fill the otherwise-idle
            # window while the first x tile DMAs in and the weights unpack,
            # and push the HAM clock gate to 8/8 (2.4 GHz) before real
            # matmuls start. (lhsT and rhs must be DIFFERENT tiles: using
            # the same SBUF region for both operands wedges the PE.)
            wdummy = cpool.tile([P, P], bf16, name="wdummy")
            nc.gpsimd.memset(wdummy[:], 0.0)
            ones3 = cpool.tile([3, P], bf16, name="ones3")
            nc.gpsimd.memset(ones3[:], 1.0)
            for _ in range(150):
                wps = pwarm.tile([P, P], f32, name="wps", tag="wps")
                nc.tensor.matmul(wps[:], lhsT=wdummy[:], rhs=ident[:],
                                 start=True, stop=True)

            # ---------------- weight prep (one-time) ----------------
            # packed u16 [nsh, K8] --transpose--> [K8, nsh] as T tiles of
            # [128, nsh]; partition p of tile t is k8 = 128*t + p.
            # lane j of u16 holds the ternary code for k = 8*k8 + j.
            w_sb = [wpool.tile([P, J, nsh], fp8, name=f"wsb{t}")
                    for t in range(T)]
            u16t = []
            for t in range(T):
                u = u16pool.tile([P, nsh], i16, name="u16t", tag="u16t")
                nc.sync.dma_start(u[:], wp_d[:, ds(P * t, P)], transpose=True)
                u16t.append(u)

            def load_quant(mi):
                # token-major tile: per-token absmax -> quant scale
                xs = xpool.tile([P, k], f32, name="xs", tag="xs")
                nc.sync.dma_start(xs[:], x_d[ds(P * mi, P), :])
                amax = spool.tile([P, 1], f32, name="amax", tag="amax")
                nc.vector.tensor_reduce(amax[:], xs[:], axis=AX.X, op=OP.max,
                                        apply_absolute_value=True)
                nc.vector.tensor_scalar_max(amax[:], amax[:], 1e-5)
                # s127 = 127 * (1/amax). No engine has an IEEE divide; can
                # differ from the reference's 127/amax by ~1 ulp, flipping
                # round(x*s) only for values within ~1 ulp of a .5 boundary
                # (a couple per million; each shifts out by one quant step).
                rcp = spool.tile([P, 1], f32, name="rcp", tag="rcp")
                nc.vector.reciprocal(rcp[:], amax[:])
                s127 = spool.tile([P, 1], f32, name="s127", tag="s127")
                nc.vector.tensor_scalar_mul(s127[:], rcp[:], 127.0)
                oscale = spool.tile([P, 1], f32, name="oscale", tag="oscale")
                nc.vector.tensor_scalar_mul(oscale[:], amax[:], wsv / 127.0)
                # Replicate s127 (a per-partition column) across partitions
                # exactly via the PE: split s into 3 bf16 parts (exact
                # Dekker-style decomposition, s = hi + mid + lo), transpose
                # the [128,3] stack with an identity matmul, then a ones-
                # matmul sums the parts in fp32 PSUM -> exact s on every
                # partition.
                s3c = spool.tile([P, 3], bf16, name="s3c", tag="s3c")
                shf = spool.tile([P, 1], f32, name="shf", tag="shf")
                r1 = spool.tile([P, 1], f32, name="r1", tag="r1")
                r2 = spool.tile([P, 1], f32, name="r2", tag="r2")
                nc.vector.tensor_copy(s3c[:, 0:1], s127[:])
                nc.vector.tensor_copy(shf[:], s3c[:, 0:1])
                nc.vector.tensor_tensor(r1[:], s127[:], shf[:], OP.subtract)
                nc.vector.tensor_copy(s3c[:, 1:2], r1[:])
                nc.vector.tensor_copy(shf[:], s3c[:, 1:2])
                nc.vector.tensor_tensor(r2[:], r1[:], shf[:], OP.subtract)
                nc.vector.tensor_copy(s3c[:, 2:3], r2[:])
                ps3 = pwarm.tile([3, P], f32, name="ps3", tag="ps3")
                nc.tensor.matmul(ps3[:], lhsT=s3c[:], rhs=ident[:],
                                 start=True, stop=True)
                s3r = spool.tile([3, P], bf16, name="s3r", tag="s3r")
                nc.vector.tensor_copy(s3r[:], ps3[:])
                psrep = pwarm.tile([P, P], f32, name="psrep", tag="wps")
                nc.tensor.matmul(psrep[:], lhsT=ones3[:], rhs=s3r[:],
                                 start=True, stop=True)
                s_rep = spool.tile([P, P], f32, name="s_rep", tag="s_rep")
                nc.vector.tensor_copy(s_rep[:], psrep[:])
                # K-major tile, quantized in place:
                # xq = RNE(xt * s) via the magic-number trick (exact fp32
                # RNE on DVE; all-integer bf16 result).
                xtf = xtfpool.tile([P, T, J, P], f32, name="xtf", tag="xtf")
                for t in range(T):
                    nc.sync.dma_start(xtf[:, t],
                                      xt_d[t, :, :, ds(P * mi, P)])
                nc.vector.tensor_tensor(
                    xtf[:], xtf[:],
                    s_rep[:, None, None, :].to_broadcast((P, T, J, P)),
                    OP.mult)
                nc.vector.tensor_scalar_add(xtf[:], xtf[:], MAGIC)
                xq = xqpool.tile([P, T, J, P], bf16, name="xq", tag="xq")
                nc.vector.tensor_scalar_add(xq[:], xtf[:], -MAGIC)
                return xq, oscale

            xq0 = load_quant(0)

            # unpack lane j in 512-wide column blocks ordered to match the
            # matmul groups' (ni, t, j) consumption: t = (u << (14-2j)) &
            # 0xC000 puts the 2-bit field at [15:14]; as int16 that is
            # 16384*decode(field), decode in {-2,-1,0,1} (two's
            # complement). The ACT scaled copy (x * 2^-14 -> fp8) finishes
            # the exact decode.
            for nb in range(nsh // NB):
                for t in range(T):
                    for j in range(J):
                        tmp = tmppool.tile([P, NB], i16, name="tmp",
                                           tag="tmp")
                        nc.vector.tensor_scalar(
                            tmp[:], u16t[t][:, ds(NB * nb, NB)],
                            14 - 2 * j, -16384,
                            OP.logical_shift_left, OP.bitwise_and)
                        nc.scalar.mul(w_sb[t][:, j, ds(NB * nb, NB)],
                                      tmp[:], 2.0 ** -14)

            # ---------------- main loop ----------------
            qcache = xq0
            for mi in range(MT):
                xq, oscale = qcache
                if mi + 1 < MT:
                    qcache = load_quant(mi + 1)
                for ni in range(NT):
                    ps = psm.tile([P, 512], f32, name="psm", tag="psm")
                    step = 0
                    for t in range(T):
                        for j in range(J):
                            nc.tensor.matmul(
                                ps[:], lhsT=xq[:, t, j, :],
                                rhs=w_sb[t][:, j, ds(512 * ni, 512)],
                                start=(step == 0), stop=(step == T * J - 1))
                            step += 1
                    ot = opool.tile([P, 512], f32, name="ot", tag="ot")
                    nc.vector.tensor_scalar_mul(ot[:], ps[:], oscale[:])
                    nc.sync.dma_start(
                        out_d[ds(P * mi, P), ds(512 * ni, 512)], ot[:])

    nc.compile()
    return nc


def _permute_xt(x2: np.ndarray, m_tokens: int, k: int) -> np.ndarray:
    """x2 [M, K] -> xt [T, 128, 8, M] with xt[t, p, j, m] = x2[m, 1024t+8p+j]
    (the K-partition order the unpacked weights live in)."""
    T = k // 1024
    xr = x2.reshape(m_tokens, T, P, 8)          # [m, t, p, j]
    return np.ascontiguousarray(xr.transpose(1, 2, 3, 0))


def kernel(x: np.ndarray, weight: np.ndarray,
           weight_scale: np.ndarray) -> np.ndarray:
    """x: [B,S,K] f32; weight: [N, K//4] uint8 packed ternary;
    weight_scale: [4] f32 (replicated scalar). Returns [B,S,N] f32."""
    _ensure_axon_hooks()
    from concourse import bass_utils

    x2 = np.ascontiguousarray(np.asarray(x).reshape(M, K), dtype=np.float32)
    xt = _permute_xt(x2, M, K)
    wp = np.ascontiguousarray(np.asarray(weight, dtype=np.uint8))
    wp16 = wp.view(np.int16)              # [N, K//8] little-endian pairs
    wsv = float(np.asarray(weight_scale).reshape(-1)[0])

    nc = build_program(wsv)
    in_maps = [
        {"x": x2, "xt": xt,
         "wp": np.ascontiguousarray(wp16[c * NSH:(c + 1) * NSH])}
        for c in range(NCORES)
    ]
    last_err = None
    for _attempt in range(3):
        try:
            res = bass_utils.run_bass_kernel_spmd(
                nc, in_maps, core_ids=list(range(NCORES)))
            break
        except Exception as e:  # transient NRT device errors recover on retry
            last_err = e
            import time as _time
            _time.sleep(5.0)
    else:
        raise last_err
    out = np.concatenate(
        [res.results[c]["out"] for c in range(NCORES)], axis=1)
    return out.reshape(B, S, N)


# revision 32
# speedup vs baseline: 1.4667x; 1.0118x over previous
"""BitLinear (BitNet b1.58) kernel for 8x Trainium2 NeuronCores.

y = (round(x * 127/absmax(x)) @ unpack_ternary(weight).T) * weight_scale / (127/absmax(x))

Strategy (column-parallel / tensor-parallel over output features N):
  - Shard packed weight rows (N) across 8 cores; replicate activations.
  - On device: unpack the 2-bit ternary weights once into resident SBUF fp8
    (two's-complement decode: exact), quantize activations per-token to
    int8-valued bf16 (exact), and run the whole GEMM in bf16xfp8 with fp32
    PSUM accumulation (exact: all products/sums are small integers < 2^24).
  - x is uploaded twice: token-major (for the per-token absmax reduce) and
    K-major pre-permuted to the weight partition order (pure host layout
    change) so no on-device transposes are needed; quantization is applied
    directly to the K-major copy with the per-token scale replicated across
    partitions by a pair of tiny DMAs.
  - Host concatenates per-core outputs along N.
"""

import sys
import types
import functools

import numpy as np

# ---------------------------------------------------------------------------
# Problem constants (hardcoded; kernel.py must be self-contained)
# ---------------------------------------------------------------------------
B, S, K, N = 2, 2048, 4096, 16384
NCORES = 8
M = B * S                  # 4096 tokens
NSH = N // NCORES          # 2048 output features per core
P = 128
MAGIC = 12582912.0         # 1.5 * 2**23: float32 round-to-nearest-even bias


def _ensure_axon_hooks():
    """The container's antenv lacks axon_hooks; synthesize it so
    run_bass_kernel_spmd(trace=True) can register the NTFF profile hook."""
    if "antenv.axon_hooks" in sys.modules:
        return
    try:
        import antenv
    except ImportError:
        return
    m = types.ModuleType("antenv.axon_hooks")
    holder = [None]
    m.set_axon_ntff_profile_hook = lambda h: holder.__setitem__(0, h)
    m.get_axon_ntff_profile_hook = lambda: holder[0]
    sys.modules["antenv.axon_hooks"] = m
    antenv.axon_hooks = m
    try:
        from trn_agent_boot.trn_boot import _ntff_profile_via_ctypes

        m.set_axon_ntff_profile_hook(
            _ntff_profile_via_ctypes("/opt/axon/libaxon_pjrt.so")
        )
    except Exception:
        pass


@functools.lru_cache(maxsize=4)
def build_program(wsv: float, m_tokens: int = M, nsh: int = NSH, k: int = K):
    """Build the single-core SPMD Bass program.

    wsv: weight_scale[0] (baked as an immediate into the output scale).
    """
    import concourse.bass as bass  # noqa: F401
    import concourse.mybir as mybir
    import concourse.tile as tile
    from concourse import bacc
    from concourse.bass import ds
    from concourse.masks import make_identity

    f32 = mybir.dt.float32
    bf16 = mybir.dt.bfloat16
    fp8 = mybir.dt.float8e4
    i16 = mybir.dt.int16
    OP = mybir.AluOpType
    AX = mybir.AxisListType

    T = k // 1024            # k8-outer tiles of 128 partitions (4)
    J = 8                    # 2-bit lanes per uint16
    K8 = k // 8              # 512
    MT = m_tokens // P       # m-tiles
    NT = nsh // 512          # n-tiles
    NB = 512                 # unpack column-block size (matches MM groups)

    nc = bacc.Bacc("TRN2", target_bir_lowering=False, debug=False,
                   num_devices=NCORES)
    x_d = nc.dram_tensor("x", [m_tokens, k], f32, kind="ExternalInput").ap()
    # K-major permuted activations: xt[t, p, j, m] = x[m, 1024t + 8p + j]
    xt_d = nc.dram_tensor("xt", [T, P, J, m_tokens], f32,
                          kind="ExternalInput").ap()
    wp_d = nc.dram_tensor("wp", [nsh, K8], i16, kind="ExternalInput").ap()
    out_d = nc.dram_tensor("out", [m_tokens, nsh], f32,
                           kind="ExternalOutput").ap()

    with tile.TileContext(nc) as tc:
        from contextlib import ExitStack

        with ExitStack() as ctx:
            cpool = ctx.enter_context(tc.tile_pool(name="const", bufs=1))
            wpool = ctx.enter_context(tc.tile_pool(name="w", bufs=1))
            u16pool = ctx.enter_context(tc.tile_pool(name="u16", bufs=4))
            tmppool = ctx.enter_context(tc.tile_pool(name="tmp", bufs=3))
            xpool = ctx.enter_context(tc.tile_pool(name="x", bufs=2))
            xtfpool = ctx.enter_context(tc.tile_pool(name="xtf", bufs=2))
            xqpool = ctx.enter_context(tc.tile_pool(name="xq", bufs=2))
            opool = ctx.enter_context(tc.tile_pool(name="o", bufs=3))
            spool = ctx.enter_context(tc.tile_pool(name="s", bufs=2))
            pwarm = ctx.enter_context(
                tc.tile_pool(name="pwarm", bufs=2, space="PSUM"))
            psm = ctx.enter_context(
                tc.tile_pool(name="psm", bufs=4, space="PSUM"))
            ps3pool = ctx.enter_context(
                tc.tile_pool(name="ps3p", bufs=1, space="PSUM"))

            ident = cpool.tile([P, P], bf16, name="ident")
            make_identity(nc, ident[:])

            # PE warmup: dependency-free matmuls fill the otherwise-idle
            # window while the first x tile DMAs in and the weights unpack,
            # and push the HAM clock gate to 8/8 (2.4 GHz) before real
            # matmuls start. (lhsT and rhs must be DIFFERENT tiles: using
            # the same SBUF region for both operands wedges the PE.)
            wdummy = cpool.tile([P, P], bf16, name="wdummy")
            nc.gpsimd.memset(wdummy[:], 0.0)
            wdummy5 = cpool.tile([P, 512], bf16, name="wdummy5")
            nc.gpsimd.memset(wdummy5[:], 0.0)
            ones3 = cpool.tile([3, P], bf16, name="ones3")
            nc.gpsimd.memset(ones3[:], 1.0)
            for _ in range(170):
                wps = pwarm.tile([P, 512], f32, name="wps", tag="wps")
                nc.tensor.matmul(wps[:], lhsT=wdummy[:], rhs=wdummy5[:],
                                 start=True, stop=True)

            # ---------------- weight prep (one-time) ----------------
            # packed u16 [nsh, K8] --transpose--> [K8, nsh] as T tiles of
            # [128, nsh]; partition p of tile t is k8 = 128*t + p.
            # lane j of u16 holds the ternary code for k = 8*k8 + j.
            w_sb = [wpool.tile([P, J, nsh], fp8, name=f"wsb{t}")
                    for t in range(T)]
            u16t = []
            for t in range(T):
                u = u16pool.tile([P, nsh], i16, name="u16t", tag="u16t")
                nc.sync.dma_start(u[:], wp_d[:, ds(P * t, P)], transpose=True)
                u16t.append(u)

            def load_quant(mi):
                # token-major tile: per-token absmax -> quant scale
                xs = xpool.tile([P, k], f32, name="xs", tag="xs")
                nc.sync.dma_start(xs[:], x_d[ds(P * mi, P), :])
                amax = spool.tile([P, 1], f32, name="amax", tag="amax")
                nc.vector.tensor_reduce(amax[:], xs[:], axis=AX.X, op=OP.max,
                                        apply_absolute_value=True)
                nc.vector.tensor_scalar_max(amax[:], amax[:], 1e-5)
                # s127 = 127 * (1/amax). No engine has an IEEE divide; can
                # differ from the reference's 127/amax by ~1 ulp, flipping
                # round(x*s) only for values within ~1 ulp of a .5 boundary
                # (a couple per million; each shifts out by one quant step).
                rcp = spool.tile([P, 1], f32, name="rcp", tag="rcp")
                nc.vector.reciprocal(rcp[:], amax[:])
                s127 = spool.tile([P, 1], f32, name="s127", tag="s127")
                nc.vector.tensor_scalar_mul(s127[:], rcp[:], 127.0)
                oscale = spool.tile([P, 1], f32, name="oscale", tag="oscale")
                nc.vector.tensor_scalar_mul(oscale[:], amax[:], wsv / 127.0)
                # Replicate s127 (a per-partition column) across partitions
                # exactly via the PE: split s into 3 bf16 parts (exact
                # Dekker-style decomposition, s = hi + mid + lo), transpose
                # the [128,3] stack with an identity matmul, then a ones-
                # matmul sums the parts in fp32 PSUM -> exact s on every
                # partition.
                s3c = spool.tile([P, 3], bf16, name="s3c", tag="s3c")
                shf = spool.tile([P, 1], f32, name="shf", tag="shf")
                r1 = spool.tile([P, 1], f32, name="r1", tag="r1")
                r2 = spool.tile([P, 1], f32, name="r2", tag="r2")
                nc.vector.tensor_copy(s3c[:, 0:1], s127[:])
                nc.vector.tensor_copy(shf[:], s3c[:, 0:1])
                nc.vector.tensor_tensor(r1[:], s127[:], shf[:], OP.subtract)
                nc.vector.tensor_copy(s3c[:, 1:2], r1[:])
                nc.vector.tensor_copy(shf[:], s3c[:, 1:2])
                nc.vector.tensor_tensor(r2[:], r1[:], shf[:], OP.subtract)
                nc.vector.tensor_copy(s3c[:, 2:3], r2[:])
                ps3 = ps3pool.tile([3, P], f32, name="ps3", tag="ps3")
                nc.tensor.matmul(ps3[:], lhsT=s3c[:], rhs=ident[:],
                                 start=True, stop=True)
                s3r = spool.tile([3, P], bf16, name="s3r", tag="s3r")
                nc.vector.tensor_copy(s3r[:], ps3[:])
                psrep = pwarm.tile([P, P], f32, name="psrep", tag="wps")
                nc.tensor.matmul(psrep[:], lhsT=ones3[:], rhs=s3r[:],
                                 start=True, stop=True)
                s_rep = spool.tile([P, P], f32, name="s_rep", tag="s_rep")
                nc.vector.tensor_copy(s_rep[:], psrep[:])
                # K-major tile, quantized in place:
                # xq = RNE(xt * s) via the magic-number trick (exact fp32
                # RNE on DVE; all-integer bf16 result).
                xtf = xtfpool.tile([P, T, J, P], f32, name="xtf", tag="xtf")
                for t in range(T):
                    nc.sync.dma_start(xtf[:, t],
                                      xt_d[t, :, :, ds(P * mi, P)])
                nc.vector.tensor_tensor(
                    xtf[:], xtf[:],
                    s_rep[:, None, None, :].to_broadcast((P, T, J, P)),
                    OP.mult)
                # (v + M) - M in one tensor_scalar: the DVE rounds op0's
                # result to fp32 before op1, which is exactly the RNE the
                # magic-number round needs (verified bit-exact vs numpy).
                xq = xqpool.tile([P, T, J, P], bf16, name="xq", tag="xq")
                nc.vector.tensor_scalar(xq[:], xtf[:], MAGIC, -MAGIC,
                                        OP.add, OP.add)
                return xq, oscale

            xq0 = load_quant(0)

            # unpack lane j in 512-wide column blocks ordered to match the
            # matmul groups' (ni, t, j) consumption: t = (u << (14-2j)) &
            # 0xC000 puts the 2-bit field at [15:14]; as int16 that is
            # 16384*decode(field), decode in {-2,-1,0,1} (two's
            # complement). The ACT scaled copy (x * 2^-14 -> fp8) finishes
            # the exact decode.
            for nb in range(nsh // NB):
                for t in range(T):
                    for j in range(J):
                        tmp = tmppool.tile([P, NB], i16, name="tmp",
                                           tag="tmp")
                        nc.vector.tensor_scalar(
                            tmp[:], u16t[t][:, ds(NB * nb, NB)],
                            14 - 2 * j, -16384,
                            OP.logical_shift_left, OP.bitwise_and)
                        nc.scalar.mul(w_sb[t][:, j, ds(NB * nb, NB)],
                                      tmp[:], 2.0 ** -14)

            # ---------------- main loop ----------------
            qcache = xq0
            for mi in range(MT):
                xq, oscale = qcache
                if mi + 1 < MT:
                    qcache = load_quant(mi + 1)
                for ni in range(NT):
                    ps = psm.tile([P, 512], f32, name="psm", tag="psm")
                    step = 0
                    for t in range(T):
                        for j in range(J):
                            nc.tensor.matmul(
                                ps[:], lhsT=xq[:, t, j, :],
                                rhs=w_sb[t][:, j, ds(512 * ni, 512)],
                                start=(step == 0), stop=(step == T * J - 1))
                            step += 1
                    ot = opool.tile([P, 512], f32, name="ot", tag="ot")
                    nc.vector.tensor_scalar_mul(ot[:], ps[:], oscale[:])
                    nc.sync.dma_start(
                        out_d[ds(P * mi, P), ds(512 * ni, 512)], ot[:])

    nc.compile()
    return nc


def _permute_xt(x2: np.ndarray, m_tokens: int, k: int) -> np.ndarray:
    """x2 [M, K] -> xt [T, 128, 8, M] with xt[t, p, j, m] = x2[m, 1024t+8p+j]
    (the K-partition order the unpacked weights live in)."""
    T = k // 1024
    xr = x2.reshape(m_tokens, T, P, 8)          # [m, t, p, j]
    return np.ascontiguousarray(xr.transpose(1, 2, 3, 0))


def kernel(x: np.ndarray, weight: np.ndarray,
           weight_scale: np.ndarray) -> np.ndarray:
    """x: [B,S,K] f32; weight: [N, K//4] uint8 packed ternary;
    weight_scale: [4] f32 (replicated scalar). Returns [B,S,N] f32."""
    _ensure_axon_hooks()
    from concourse import bass_utils

    x2 = np.ascontiguousarray(np.asarray(x).reshape(M, K), dtype=np.float32)
    xt = _permute_xt(x2, M, K)
    wp = np.ascontiguousarray(np.asarray(weight, dtype=np.uint8))
    wp16 = wp.view(np.int16)              # [N, K//8] little-endian pairs
    wsv = float(np.asarray(weight_scale).reshape(-1)[0])

    nc = build_program(wsv)
    in_maps = [
        {"x": x2, "xt": xt,
         "wp": np.ascontiguousarray(wp16[c * NSH:(c + 1) * NSH])}
        for c in range(NCORES)
    ]
    last_err = None
    for _attempt in range(3):
        try:
            res = bass_utils.run_bass_kernel_spmd(
                nc, in_maps, core_ids=list(range(NCORES)))
            break
        except Exception as e:  # transient NRT device errors recover on retry
            last_err = e
            import time as _time
            _time.sleep(5.0)
    else:
        raise last_err
    out = np.concatenate(
        [res.results[c]["out"] for c in range(NCORES)], axis=1)
    return out.reshape(B, S, N)


# revision 35
# speedup vs baseline: 1.4683x; 1.0011x over previous
"""BitLinear (BitNet b1.58) kernel for 8x Trainium2 NeuronCores.

y = (round(x * 127/absmax(x)) @ unpack_ternary(weight).T) * weight_scale / (127/absmax(x))

Strategy (column-parallel / tensor-parallel over output features N):
  - Shard packed weight rows (N) across 8 cores; replicate activations.
  - On device: unpack the 2-bit ternary weights once into resident SBUF fp8
    (two's-complement decode: exact), quantize activations per-token to
    int8-valued bf16 (exact), and run the whole GEMM in bf16xfp8 with fp32
    PSUM accumulation (exact: all products/sums are small integers < 2^24).
  - x is uploaded twice: token-major (for the per-token absmax reduce) and
    K-major pre-permuted to the weight partition order (pure host layout
    change) so no on-device transposes are needed; quantization is applied
    directly to the K-major copy with the per-token scale replicated across
    partitions by a pair of tiny DMAs.
  - Host concatenates per-core outputs along N.
"""

import sys
import types
import functools

import numpy as np

# ---------------------------------------------------------------------------
# Problem constants (hardcoded; kernel.py must be self-contained)
# ---------------------------------------------------------------------------
B, S, K, N = 2, 2048, 4096, 16384
NCORES = 8
M = B * S                  # 4096 tokens
NSH = N // NCORES          # 2048 output features per core
P = 128
MAGIC = 12582912.0         # 1.5 * 2**23: float32 round-to-nearest-even bias


def _ensure_axon_hooks():
    """The container's antenv lacks axon_hooks; synthesize it so
    run_bass_kernel_spmd(trace=True) can register the NTFF profile hook."""
    if "antenv.axon_hooks" in sys.modules:
        return
    try:
        import antenv
    except ImportError:
        return
    m = types.ModuleType("antenv.axon_hooks")
    holder = [None]
    m.set_axon_ntff_profile_hook = lambda h: holder.__setitem__(0, h)
    m.get_axon_ntff_profile_hook = lambda: holder[0]
    sys.modules["antenv.axon_hooks"] = m
    antenv.axon_hooks = m
    try:
        from trn_agent_boot.trn_boot import _ntff_profile_via_ctypes

        m.set_axon_ntff_profile_hook(
            _ntff_profile_via_ctypes("/opt/axon/libaxon_pjrt.so")
        )
    except Exception:
        pass


@functools.lru_cache(maxsize=4)
def build_program(wsv: float, m_tokens: int = M, nsh: int = NSH, k: int = K):
    """Build the single-core SPMD Bass program.

    wsv: weight_scale[0] (baked as an immediate into the output scale).
    """
    import concourse.bass as bass  # noqa: F401
    import concourse.mybir as mybir
    import concourse.tile as tile
    from concourse import bacc
    from concourse.bass import ds
    from concourse.masks import make_identity

    f32 = mybir.dt.float32
    bf16 = mybir.dt.bfloat16
    fp8 = mybir.dt.float8e4
    i16 = mybir.dt.int16
    OP = mybir.AluOpType
    AX = mybir.AxisListType

    T = k // 1024            # k8-outer tiles of 128 partitions (4)
    J = 8                    # 2-bit lanes per uint16
    K8 = k // 8              # 512
    MT = m_tokens // P       # m-tiles
    NT = nsh // 512          # n-tiles
    NB = 512                 # unpack column-block size (matches MM groups)

    nc = bacc.Bacc("TRN2", target_bir_lowering=False, debug=False,
                   num_devices=NCORES)
    x_d = nc.dram_tensor("x", [m_tokens, k], f32, kind="ExternalInput").ap()
    # K-major permuted activations: xt[t, p, j, m] = x[m, 1024t + 8p + j]
    xt_d = nc.dram_tensor("xt", [T, P, J, m_tokens], f32,
                          kind="ExternalInput").ap()
    wp_d = nc.dram_tensor("wp", [nsh, K8], i16, kind="ExternalInput").ap()
    out_d = nc.dram_tensor("out", [m_tokens, nsh], f32,
                           kind="ExternalOutput").ap()

    with tile.TileContext(nc) as tc:
        from contextlib import ExitStack

        with ExitStack() as ctx:
            cpool = ctx.enter_context(tc.tile_pool(name="const", bufs=1))
            wpool = ctx.enter_context(tc.tile_pool(name="w", bufs=1))
            u16pool = ctx.enter_context(tc.tile_pool(name="u16", bufs=4))
            tmppool = ctx.enter_context(tc.tile_pool(name="tmp", bufs=3))
            xpool = ctx.enter_context(tc.tile_pool(name="x", bufs=2))
            xtfpool = ctx.enter_context(tc.tile_pool(name="xtf", bufs=2))
            xqpool = ctx.enter_context(tc.tile_pool(name="xq", bufs=3))
            opool = ctx.enter_context(tc.tile_pool(name="o", bufs=3))
            spool = ctx.enter_context(tc.tile_pool(name="s", bufs=2))
            pwarm = ctx.enter_context(
                tc.tile_pool(name="pwarm", bufs=2, space="PSUM"))
            psm = ctx.enter_context(
                tc.tile_pool(name="psm", bufs=4, space="PSUM"))
            ps3pool = ctx.enter_context(
                tc.tile_pool(name="ps3p", bufs=1, space="PSUM"))

            ident = cpool.tile([P, P], bf16, name="ident")
            make_identity(nc, ident[:])

            # PE warmup: dependency-free matmuls fill the otherwise-idle
            # window while the first x tile DMAs in and the weights unpack,
            # and push the HAM clock gate to 8/8 (2.4 GHz) before real
            # matmuls start. (lhsT and rhs must be DIFFERENT tiles: using
            # the same SBUF region for both operands wedges the PE.)
            wdummy = cpool.tile([P, P], bf16, name="wdummy")
            nc.gpsimd.memset(wdummy[:], 0.0)
            wdummy5 = cpool.tile([P, 512], bf16, name="wdummy5")
            nc.gpsimd.memset(wdummy5[:], 0.0)
            ones3 = cpool.tile([3, P], bf16, name="ones3")
            nc.gpsimd.memset(ones3[:], 1.0)
            for _ in range(170):
                wps = pwarm.tile([P, 512], f32, name="wps", tag="wps")
                nc.tensor.matmul(wps[:], lhsT=wdummy[:], rhs=wdummy5[:],
                                 start=True, stop=True)

            # ---------------- weight prep (one-time) ----------------
            # packed u16 [nsh, K8] --transpose--> [K8, nsh] as T tiles of
            # [128, nsh]; partition p of tile t is k8 = 128*t + p.
            # lane j of u16 holds the ternary code for k = 8*k8 + j.
            w_sb = [wpool.tile([P, J, nsh], fp8, name=f"wsb{t}")
                    for t in range(T)]
            u16t = []
            for t in range(T):
                u = u16pool.tile([P, nsh], i16, name="u16t", tag="u16t")
                nc.sync.dma_start(u[:], wp_d[:, ds(P * t, P)], transpose=True)
                u16t.append(u)

            def load_quant(mi):
                # token-major tile: per-token absmax -> quant scale
                xs = xpool.tile([P, k], f32, name="xs", tag="xs")
                nc.sync.dma_start(xs[:], x_d[ds(P * mi, P), :])
                amax = spool.tile([P, 1], f32, name="amax", tag="amax")
                nc.vector.tensor_reduce(amax[:], xs[:], axis=AX.X, op=OP.max,
                                        apply_absolute_value=True)
                nc.vector.tensor_scalar_max(amax[:], amax[:], 1e-5)
                # s127 = 127 * (1/amax). No engine has an IEEE divide; can
                # differ from the reference's 127/amax by ~1 ulp, flipping
                # round(x*s) only for values within ~1 ulp of a .5 boundary
                # (a couple per million; each shifts out by one quant step).
                rcp = spool.tile([P, 1], f32, name="rcp", tag="rcp")
                nc.vector.reciprocal(rcp[:], amax[:])
                s127 = spool.tile([P, 1], f32, name="s127", tag="s127")
                nc.vector.tensor_scalar_mul(s127[:], rcp[:], 127.0)
                oscale = spool.tile([P, 1], f32, name="oscale", tag="oscale")
                nc.vector.tensor_scalar_mul(oscale[:], amax[:], wsv / 127.0)
                # Replicate s127 (a per-partition column) across partitions
                # exactly via the PE: split s into 3 bf16 parts (exact
                # Dekker-style decomposition, s = hi + mid + lo), transpose
                # the [128,3] stack with an identity matmul, then a ones-
                # matmul sums the parts in fp32 PSUM -> exact s on every
                # partition.
                s3c = spool.tile([P, 3], bf16, name="s3c", tag="s3c")
                shf = spool.tile([P, 1], f32, name="shf", tag="shf")
                r1 = spool.tile([P, 1], f32, name="r1", tag="r1")
                r2 = spool.tile([P, 1], f32, name="r2", tag="r2")
                nc.vector.tensor_copy(s3c[:, 0:1], s127[:])
                nc.vector.tensor_copy(shf[:], s3c[:, 0:1])
                nc.vector.tensor_tensor(r1[:], s127[:], shf[:], OP.subtract)
                nc.vector.tensor_copy(s3c[:, 1:2], r1[:])
                nc.vector.tensor_copy(shf[:], s3c[:, 1:2])
                nc.vector.tensor_tensor(r2[:], r1[:], shf[:], OP.subtract)
                nc.vector.tensor_copy(s3c[:, 2:3], r2[:])
                ps3 = ps3pool.tile([3, P], f32, name="ps3", tag="ps3")
                nc.tensor.matmul(ps3[:], lhsT=s3c[:], rhs=ident[:],
                                 start=True, stop=True)
                s3r = spool.tile([3, P], bf16, name="s3r", tag="s3r")
                nc.vector.tensor_copy(s3r[:], ps3[:])
                psrep = pwarm.tile([P, P], f32, name="psrep", tag="wps")
                nc.tensor.matmul(psrep[:], lhsT=ones3[:], rhs=s3r[:],
                                 start=True, stop=True)
                s_rep = spool.tile([P, P], f32, name="s_rep", tag="s_rep")
                nc.vector.tensor_copy(s_rep[:], psrep[:])
                # K-major tile, quantized in place:
                # xq = RNE(xt * s) via the magic-number trick (exact fp32
                # RNE on DVE; all-integer bf16 result).
                xtf = xtfpool.tile([P, T, J, P], f32, name="xtf", tag="xtf")
                for t in range(T):
                    nc.sync.dma_start(xtf[:, t],
                                      xt_d[t, :, :, ds(P * mi, P)])
                nc.vector.tensor_tensor(
                    xtf[:], xtf[:],
                    s_rep[:, None, None, :].to_broadcast((P, T, J, P)),
                    OP.mult)
                # (v + M) - M in one tensor_scalar: the DVE rounds op0's
                # result to fp32 before op1, which is exactly the RNE the
                # magic-number round needs (verified bit-exact vs numpy).
                xq = xqpool.tile([P, T, J, P], bf16, name="xq", tag="xq")
                nc.vector.tensor_scalar(xq[:], xtf[:], MAGIC, -MAGIC,
                                        OP.add, OP.add)
                return xq, oscale

            # unpack lane j in 512-wide column blocks ordered to match the
            # matmul groups' (ni, t, j) consumption: t = (u << (14-2j)) &
            # 0xC000 puts the 2-bit field at [15:14]; as int16 that is
            # 16384*decode(field), decode in {-2,-1,0,1} (two's
            # complement). The ACT scaled copy (x * 2^-14 -> fp8) finishes
            # the exact decode. Interleave the first two token-tile quant
            # chains between blocks so the PE can start real matmuls early.
            def unpack_block(nb):
                for t in range(T):
                    for j in range(J):
                        tmp = tmppool.tile([P, NB], i16, name="tmp",
                                           tag="tmp")
                        nc.vector.tensor_scalar(
                            tmp[:], u16t[t][:, ds(NB * nb, NB)],
                            14 - 2 * j, -16384,
                            OP.logical_shift_left, OP.bitwise_and)
                        nc.scalar.mul(w_sb[t][:, j, ds(NB * nb, NB)],
                                      tmp[:], 2.0 ** -14)

            NBLK = nsh // NB
            unpack_block(0)
            qd = {0: load_quant(0)}
            if NBLK > 1:
                unpack_block(1)
            if MT > 1:
                qd[1] = load_quant(1)
            for nb in range(2, NBLK):
                unpack_block(nb)

            # ---------------- main loop ----------------
            for mi in range(MT):
                xq, oscale = qd.pop(mi)
                if mi + 2 < MT:
                    qd[mi + 2] = load_quant(mi + 2)
                for ni in range(NT):
                    ps = psm.tile([P, 512], f32, name="psm", tag="psm")
                    step = 0
                    for t in range(T):
                        for j in range(J):
                            nc.tensor.matmul(
                                ps[:], lhsT=xq[:, t, j, :],
                                rhs=w_sb[t][:, j, ds(512 * ni, 512)],
                                start=(step == 0), stop=(step == T * J - 1))
                            step += 1
                    ot = opool.tile([P, 512], f32, name="ot", tag="ot")
                    nc.vector.tensor_scalar_mul(ot[:], ps[:], oscale[:])
                    nc.sync.dma_start(
                        out_d[ds(P * mi, P), ds(512 * ni, 512)], ot[:])

    nc.compile()
    return nc


def _permute_xt(x2: np.ndarray, m_tokens: int, k: int) -> np.ndarray:
    """x2 [M, K] -> xt [T, 128, 8, M] with xt[t, p, j, m] = x2[m, 1024t+8p+j]
    (the K-partition order the unpacked weights live in)."""
    T = k // 1024
    xr = x2.reshape(m_tokens, T, P, 8)          # [m, t, p, j]
    return np.ascontiguousarray(xr.transpose(1, 2, 3, 0))


def kernel(x: np.ndarray, weight: np.ndarray,
           weight_scale: np.ndarray) -> np.ndarray:
    """x: [B,S,K] f32; weight: [N, K//4] uint8 packed ternary;
    weight_scale: [4] f32 (replicated scalar). Returns [B,S,N] f32."""
    _ensure_axon_hooks()
    from concourse import bass_utils

    x2 = np.ascontiguousarray(np.asarray(x).reshape(M, K), dtype=np.float32)
    xt = _permute_xt(x2, M, K)
    wp = np.ascontiguousarray(np.asarray(weight, dtype=np.uint8))
    wp16 = wp.view(np.int16)              # [N, K//8] little-endian pairs
    wsv = float(np.asarray(weight_scale).reshape(-1)[0])

    nc = build_program(wsv)
    in_maps = [
        {"x": x2, "xt": xt,
         "wp": np.ascontiguousarray(wp16[c * NSH:(c + 1) * NSH])}
        for c in range(NCORES)
    ]
    last_err = None
    for _attempt in range(3):
        try:
            res = bass_utils.run_bass_kernel_spmd(
                nc, in_maps, core_ids=list(range(NCORES)))
            break
        except Exception as e:  # transient NRT device errors recover on retry
            last_err = e
            import time as _time
            _time.sleep(5.0)
    else:
        raise last_err
    out = np.concatenate(
        [res.results[c]["out"] for c in range(NCORES)], axis=1)
    return out.reshape(B, S, N)


# revision 38
# speedup vs baseline: 1.4808x; 1.0085x over previous
"""BitLinear (BitNet b1.58) kernel for 8x Trainium2 NeuronCores.

y = (round(x * 127/absmax(x)) @ unpack_ternary(weight).T) * weight_scale / (127/absmax(x))

Strategy (column-parallel / tensor-parallel over output features N):
  - Shard packed weight rows (N) across 8 cores; replicate activations.
  - On device: unpack the 2-bit ternary weights once into resident SBUF fp8
    (two's-complement decode: exact), quantize activations per-token to
    int8-valued bf16 (exact), and run the whole GEMM in bf16xfp8 with fp32
    PSUM accumulation (exact: all products/sums are small integers < 2^24).
  - x is uploaded twice: token-major (for the per-token absmax reduce) and
    K-major pre-permuted to the weight partition order (pure host layout
    change) so no on-device transposes are needed; quantization is applied
    directly to the K-major copy with the per-token scale replicated across
    partitions by a pair of tiny DMAs.
  - Host concatenates per-core outputs along N.
"""

import sys
import types
import functools

import numpy as np

# ---------------------------------------------------------------------------
# Problem constants (hardcoded; kernel.py must be self-contained)
# ---------------------------------------------------------------------------
B, S, K, N = 2, 2048, 4096, 16384
NCORES = 8
M = B * S                  # 4096 tokens
NSH = N // NCORES          # 2048 output features per core
P = 128
MAGIC = 12582912.0         # 1.5 * 2**23: float32 round-to-nearest-even bias


def _ensure_axon_hooks():
    """The container's antenv lacks axon_hooks; synthesize it so
    run_bass_kernel_spmd(trace=True) can register the NTFF profile hook."""
    if "antenv.axon_hooks" in sys.modules:
        return
    try:
        import antenv
    except ImportError:
        return
    m = types.ModuleType("antenv.axon_hooks")
    holder = [None]
    m.set_axon_ntff_profile_hook = lambda h: holder.__setitem__(0, h)
    m.get_axon_ntff_profile_hook = lambda: holder[0]
    sys.modules["antenv.axon_hooks"] = m
    antenv.axon_hooks = m
    try:
        from trn_agent_boot.trn_boot import _ntff_profile_via_ctypes

        m.set_axon_ntff_profile_hook(
            _ntff_profile_via_ctypes("/opt/axon/libaxon_pjrt.so")
        )
    except Exception:
        pass


@functools.lru_cache(maxsize=4)
def build_program(wsv: float, m_tokens: int = M, nsh: int = NSH, k: int = K):
    """Build the single-core SPMD Bass program.

    wsv: weight_scale[0] (baked as an immediate into the output scale).
    """
    import concourse.bass as bass  # noqa: F401
    import concourse.mybir as mybir
    import concourse.tile as tile
    from concourse import bacc
    from concourse.bass import ds
    from concourse.masks import make_identity

    f32 = mybir.dt.float32
    bf16 = mybir.dt.bfloat16
    fp8 = mybir.dt.float8e4
    i16 = mybir.dt.int16
    OP = mybir.AluOpType
    AX = mybir.AxisListType

    T = k // 1024            # k8-outer tiles of 128 partitions (4)
    J = 8                    # 2-bit lanes per uint16
    K8 = k // 8              # 512
    MT = m_tokens // P       # m-tiles
    NT = nsh // 512          # n-tiles
    NB = 512                 # unpack column-block size (matches MM groups)

    nc = bacc.Bacc("TRN2", target_bir_lowering=False, debug=False,
                   num_devices=NCORES)
    x_d = nc.dram_tensor("x", [m_tokens, k], f32, kind="ExternalInput").ap()
    # K-major permuted activations: xt[t, p, j, m] = x[m, 1024t + 8p + j]
    xt_d = nc.dram_tensor("xt", [T, P, J, m_tokens], f32,
                          kind="ExternalInput").ap()
    wp_d = nc.dram_tensor("wp", [nsh, K8], i16, kind="ExternalInput").ap()
    out_d = nc.dram_tensor("out", [m_tokens, nsh], f32,
                           kind="ExternalOutput").ap()

    with tile.TileContext(nc) as tc:
        from contextlib import ExitStack

        with ExitStack() as ctx:
            cpool = ctx.enter_context(tc.tile_pool(name="const", bufs=1))
            wpool = ctx.enter_context(tc.tile_pool(name="w", bufs=1))
            u16pool = ctx.enter_context(tc.tile_pool(name="u16", bufs=4))
            tmppool = ctx.enter_context(tc.tile_pool(name="tmp", bufs=3))
            xpool = ctx.enter_context(tc.tile_pool(name="x", bufs=2))
            xtfpool = ctx.enter_context(tc.tile_pool(name="xtf", bufs=2))
            xqpool = ctx.enter_context(tc.tile_pool(name="xq", bufs=4))
            opool = ctx.enter_context(tc.tile_pool(name="o", bufs=3))
            spool = ctx.enter_context(tc.tile_pool(name="s", bufs=2))
            pwarm = ctx.enter_context(
                tc.tile_pool(name="pwarm", bufs=2, space="PSUM"))
            psm = ctx.enter_context(
                tc.tile_pool(name="psm", bufs=4, space="PSUM"))
            ps3pool = ctx.enter_context(
                tc.tile_pool(name="ps3p", bufs=1, space="PSUM"))

            ident = cpool.tile([P, P], bf16, name="ident")
            make_identity(nc, ident[:])

            # PE warmup: dependency-free matmuls fill the otherwise-idle
            # window while the first x tile DMAs in and the weights unpack,
            # and push the HAM clock gate to 8/8 (2.4 GHz) before real
            # matmuls start. (lhsT and rhs must be DIFFERENT tiles: using
            # the same SBUF region for both operands wedges the PE.)
            wdummy = cpool.tile([P, P], bf16, name="wdummy")
            nc.gpsimd.memset(wdummy[:], 0.0)
            wdummy5 = cpool.tile([P, 512], bf16, name="wdummy5")
            nc.gpsimd.memset(wdummy5[:], 0.0)
            ones3 = cpool.tile([3, P], bf16, name="ones3")
            nc.gpsimd.memset(ones3[:], 1.0)
            for _ in range(60):
                wps = pwarm.tile([P, 512], f32, name="wps", tag="wps")
                nc.tensor.matmul(wps[:], lhsT=wdummy[:], rhs=wdummy5[:],
                                 start=True, stop=True)

            # ---------------- weight prep (one-time) ----------------
            # packed u16 [nsh, K8] --transpose--> [K8, nsh] as T tiles of
            # [128, nsh]; partition p of tile t is k8 = 128*t + p.
            # lane j of u16 holds the ternary code for k = 8*k8 + j.
            w_sb = [wpool.tile([P, J, nsh], fp8, name=f"wsb{t}")
                    for t in range(T)]
            u16t = []
            for t in range(T):
                u = u16pool.tile([P, nsh], i16, name="u16t", tag="u16t")
                nc.sync.dma_start(u[:], wp_d[:, ds(P * t, P)], transpose=True)
                u16t.append(u)

            def load_quant(mi):
                # token-major tile: per-token absmax -> quant scale
                xs = xpool.tile([P, k], f32, name="xs", tag="xs")
                nc.sync.dma_start(xs[:], x_d[ds(P * mi, P), :])
                amax = spool.tile([P, 1], f32, name="amax", tag="amax")
                nc.vector.tensor_reduce(amax[:], xs[:], axis=AX.X, op=OP.max,
                                        apply_absolute_value=True)
                nc.vector.tensor_scalar_max(amax[:], amax[:], 1e-5)
                # s127 = 127 * (1/amax). No engine has an IEEE divide; can
                # differ from the reference's 127/amax by ~1 ulp, flipping
                # round(x*s) only for values within ~1 ulp of a .5 boundary
                # (a couple per million; each shifts out by one quant step).
                rcp = spool.tile([P, 1], f32, name="rcp", tag="rcp")
                nc.vector.reciprocal(rcp[:], amax[:])
                s127 = spool.tile([P, 1], f32, name="s127", tag="s127")
                nc.vector.tensor_scalar_mul(s127[:], rcp[:], 127.0)
                oscale = spool.tile([P, 1], f32, name="oscale", tag="oscale")
                nc.vector.tensor_scalar_mul(oscale[:], amax[:], wsv / 127.0)
                # Replicate s127 (a per-partition column) across partitions
                # exactly via the PE: split s into 3 bf16 parts (exact
                # Dekker-style decomposition, s = hi + mid + lo), transpose
                # the [128,3] stack with an identity matmul, then a ones-
                # matmul sums the parts in fp32 PSUM -> exact s on every
                # partition.
                s3c = spool.tile([P, 3], bf16, name="s3c", tag="s3c")
                shf = spool.tile([P, 1], f32, name="shf", tag="shf")
                r1 = spool.tile([P, 1], f32, name="r1", tag="r1")
                r2 = spool.tile([P, 1], f32, name="r2", tag="r2")
                nc.vector.tensor_copy(s3c[:, 0:1], s127[:])
                nc.vector.tensor_copy(shf[:], s3c[:, 0:1])
                nc.vector.tensor_tensor(r1[:], s127[:], shf[:], OP.subtract)
                nc.vector.tensor_copy(s3c[:, 1:2], r1[:])
                nc.vector.tensor_copy(shf[:], s3c[:, 1:2])
                nc.vector.tensor_tensor(r2[:], r1[:], shf[:], OP.subtract)
                nc.vector.tensor_copy(s3c[:, 2:3], r2[:])
                ps3 = ps3pool.tile([3, P], f32, name="ps3", tag="ps3")
                nc.tensor.matmul(ps3[:], lhsT=s3c[:], rhs=ident[:],
                                 start=True, stop=True)
                s3r = spool.tile([3, P], bf16, name="s3r", tag="s3r")
                nc.vector.tensor_copy(s3r[:], ps3[:])
                psrep = pwarm.tile([P, P], f32, name="psrep", tag="wps")
                nc.tensor.matmul(psrep[:], lhsT=ones3[:], rhs=s3r[:],
                                 start=True, stop=True)
                s_rep = spool.tile([P, P], f32, name="s_rep", tag="s_rep")
                nc.vector.tensor_copy(s_rep[:], psrep[:])
                # K-major tile, quantized in place:
                # xq = RNE(xt * s) via the magic-number trick (exact fp32
                # RNE on DVE; all-integer bf16 result).
                xtf = xtfpool.tile([P, T, J, P], f32, name="xtf", tag="xtf")
                for t in range(T):
                    nc.sync.dma_start(xtf[:, t],
                                      xt_d[t, :, :, ds(P * mi, P)])
                nc.vector.tensor_tensor(
                    xtf[:], xtf[:],
                    s_rep[:, None, None, :].to_broadcast((P, T, J, P)),
                    OP.mult)
                # (v + M) - M in one tensor_scalar: the DVE rounds op0's
                # result to fp32 before op1, which is exactly the RNE the
                # magic-number round needs (verified bit-exact vs numpy).
                xq = xqpool.tile([P, T, J, P], bf16, name="xq", tag="xq")
                nc.vector.tensor_scalar(xq[:], xtf[:], MAGIC, -MAGIC,
                                        OP.add, OP.add)
                return xq, oscale

            # unpack lane j in 512-wide column blocks ordered to match the
            # matmul groups' (ni, t, j) consumption: t = (u << (14-2j)) &
            # 0xC000 puts the 2-bit field at [15:14]; as int16 that is
            # 16384*decode(field), decode in {-2,-1,0,1} (two's
            # complement). The ACT scaled copy (x * 2^-14 -> fp8) finishes
            # the exact decode. Interleave the first two token-tile quant
            # chains between blocks so the PE can start real matmuls early.
            def unpack_block(nb):
                for t in range(T):
                    for j in range(J):
                        tmp = tmppool.tile([P, NB], i16, name="tmp",
                                           tag="tmp")
                        nc.vector.tensor_scalar(
                            tmp[:], u16t[t][:, ds(NB * nb, NB)],
                            14 - 2 * j, -16384,
                            OP.logical_shift_left, OP.bitwise_and)
                        nc.scalar.mul(w_sb[t][:, j, ds(NB * nb, NB)],
                                      tmp[:], 2.0 ** -14)

            def emit_group(mi, ni, xq, oscale):
                ps = psm.tile([P, 512], f32, name="psm", tag="psm")
                step = 0
                for t in range(T):
                    for j in range(J):
                        nc.tensor.matmul(
                            ps[:], lhsT=xq[:, t, j, :],
                            rhs=w_sb[t][:, j, ds(512 * ni, 512)],
                            start=(step == 0), stop=(step == T * J - 1))
                        step += 1
                ot = opool.tile([P, 512], f32, name="ot", tag="ot")
                nc.vector.tensor_scalar_mul(ot[:], ps[:], oscale[:])
                nc.sync.dma_start(
                    out_d[ds(P * mi, P), ds(512 * ni, 512)], ot[:])

            NBLK = nsh // NB
            NHEAD = min(2, MT)  # striped head m-tiles
            unpack_block(0)
            qd = {mi: load_quant(mi) for mi in range(NHEAD)}
            for nb in range(1, NBLK):
                unpack_block(nb)

            # ---------------- main loop ----------------
            # Head: iterate n-stripes over the first two m-tiles so each
            # stripe only needs one just-unpacked 512-column weight block —
            # the PE hits full rate while the rest of the unpack streams.
            for ni in range(NT):
                for mi in range(NHEAD):
                    xq, oscale = qd[mi]
                    emit_group(mi, ni, xq, oscale)
            for mi in range(NHEAD, min(NHEAD + 2, MT)):
                qd[mi] = load_quant(mi)
            for mi in range(NHEAD, MT):
                xq, oscale = qd.pop(mi)
                if mi + 2 < MT:
                    qd[mi + 2] = load_quant(mi + 2)
                for ni in range(NT):
                    emit_group(mi, ni, xq, oscale)

    nc.compile()
    return nc


def _permute_xt(x2: np.ndarray, m_tokens: int, k: int) -> np.ndarray:
    """x2 [M, K] -> xt [T, 128, 8, M] with xt[t, p, j, m] = x2[m, 1024t+8p+j]
    (the K-partition order the unpacked weights live in)."""
    T = k // 1024
    xr = x2.reshape(m_tokens, T, P, 8)          # [m, t, p, j]
    return np.ascontiguousarray(xr.transpose(1, 2, 3, 0))


def kernel(x: np.ndarray, weight: np.ndarray,
           weight_scale: np.ndarray) -> np.ndarray:
    """x: [B,S,K] f32; weight: [N, K//4] uint8 packed ternary;
    weight_scale: [4] f32 (replicated scalar). Returns [B,S,N] f32."""
    _ensure_axon_hooks()
    from concourse import bass_utils

    x2 = np.ascontiguousarray(np.asarray(x).reshape(M, K), dtype=np.float32)
    xt = _permute_xt(x2, M, K)
    wp = np.ascontiguousarray(np.asarray(weight, dtype=np.uint8))
    wp16 = wp.view(np.int16)              # [N, K//8] little-endian pairs
    wsv = float(np.asarray(weight_scale).reshape(-1)[0])

    nc = build_program(wsv)
    in_maps = [
        {"x": x2, "xt": xt,
         "wp": np.ascontiguousarray(wp16[c * NSH:(c + 1) * NSH])}
        for c in range(NCORES)
    ]
    last_err = None
    for _attempt in range(3):
        try:
            res = bass_utils.run_bass_kernel_spmd(
                nc, in_maps, core_ids=list(range(NCORES)))
            break
        except Exception as e:  # transient NRT device errors recover on retry
            last_err = e
            import time as _time
            _time.sleep(5.0)
    else:
        raise last_err
    out = np.concatenate(
        [res.results[c]["out"] for c in range(NCORES)], axis=1)
    return out.reshape(B, S, N)
